# revision 61
# baseline (speedup 1.0000x reference)
"""Multi-head attention (B=2, L=2048, D=1024, H=16) on 8 TRN2 NeuronCores.

Sharding: batch (2) x head-group (4 heads each) = 8 shards.
Each core computes q/k/v projections for its 4 heads, attention, and a
partial output projection (its 256 rows of Wo); host sums the 4 partials
per batch and adds bo.

Device dataflow (per core):
  inputs (host-prepped, f16 except memory-bias):
    qT  [1024, 2048]  = query[b].T          (d on partitions for matmul)
    mT  [1024, 2048]  = memory[b].T
    wq  [1024, 256]   = Wq[:, J] * 0.125    (scale folded)
    wk, wv [1024, 256];  bq*0.125, bk, bv [1, 256]
    wo  [256, 1024]   = Wo[J, :]
    mb  [128, 16]     = memory_bias[b] chunked (f32, t on partitions)
  phase 1: qT_h [j, f], kT_h [j, t] (head-dim on partitions), v [t, j]
           (natural) with a constant ones denominator column per head:
           softmax(s + b) @ v == (exp(s + b) @ [v | 1]) split / last-col,
           where the bias b enters through the activation's per-partition
           bias operand (t is on partitions in the logits tile).
  phase 2: one PSUM tile [128, 2*FB] per t-chunk holds both heads of a
           pair side by side; the two K=64 logits matmuls are row-tiled
           (tile_position rows 0-63 / 64-127) so they stream through the
           PE array concurrently (the pair fuses into ~one matmul time).
           exp runs on ScalarE with the memory bias applied through the
           activation's per-partition bias operand.  AV matmuls are
           split-K row-tiled pairs (t rows 0-63 / 64-127 of each chunk)
           accumulating into two PSUM column halves; the normalization
           merges the halves on the DVE, computes both heads' approx
           reciprocals, broadcasts them with two col-tiled K=1 matmuls
           into one PSUM tile, and normalizes the whole head pair with
           a single tensor_tensor.  Out-projection pieces are held in
           reserve to fill the tail while the last finish chains drain.
"""

import numpy as np

import concourse.bass as bass
import concourse.tile as tile
from concourse import bacc, mybir
from concourse import bass_utils
from concourse.bass import ts, ds

F32 = mybir.dt.float32
I16 = mybir.dt.int16
BF16 = mybir.dt.bfloat16
FP16 = mybir.dt.float16

B, LQ, LM, D, H = 2, 2048, 2048, 1024, 16
DH = 64
HPC = 4            # heads per core
JC = HPC * DH      # 256 projection cols per core
NCORE = 8
P = 128
FB = 512           # f-block width
NFB = LQ // FB     # 4
NTC = LM // P      # 16 t-chunks
KD = D // P        # 8 contraction chunks for projections
G = 2              # t-chunks per logits step

VW = DH + 1        # v columns per head incl. denominator column
VS = DH + 2        # v column stride per head (4B alignment in f16)

# Schraudolph f16 exp constants: bits = round(z * SCH_A + SCH_B + SCH_A * b)
SCH_A = 1024.0 / float(np.log(2.0))
SCH_B = 15.0 * 1024.0 - 59.297


def sch_offload(fb, tc):
    """Which exp tiles run on the DVE instead of ScalarE."""
    # With the PE as the bottleneck the ScalarE has slack, and a DVE exp
    # (deep in-order queue) only delays the pls-slot recycling; keep all
    # exp on the ScalarE.
    return False


def build_kernel(mm_dt=FP16, with_biases=False):
    MM = mm_dt
    nc = bacc.Bacc("TRN2", target_bir_lowering=False, debug=False)

    qTd = nc.dram_tensor("qT", [D, LQ], MM, kind="ExternalInput").ap()
    mTd = nc.dram_tensor("mT", [D, LM], MM, kind="ExternalInput").ap()
    wqd = nc.dram_tensor("wq", [D, JC], MM, kind="ExternalInput").ap()
    wkd = nc.dram_tensor("wk", [D, JC], MM, kind="ExternalInput").ap()
    wvd = nc.dram_tensor("wv", [D, JC], MM, kind="ExternalInput").ap()
    if with_biases:
        bqd = nc.dram_tensor("bq", [1, JC], MM, kind="ExternalInput").ap()
        bkd = nc.dram_tensor("bk", [1, JC], MM, kind="ExternalInput").ap()
        bvd = nc.dram_tensor("bv", [1, JC], MM, kind="ExternalInput").ap()
    wod = nc.dram_tensor("wo", [JC, D], MM, kind="ExternalInput").ap()
    mbd = nc.dram_tensor("mb", [P, NTC], F32, kind="ExternalInput").ap()
    outd = nc.dram_tensor("out", [LQ, D], MM, kind="ExternalOutput").ap()

    with tile.TileContext(nc) as tc:
        with (
            tc.tile_pool(name="persist", bufs=1) as persist,
            tc.tile_pool(name="vpool", bufs=1) as vpool,
            tc.tile_pool(name="consts", bufs=1) as consts,
            tc.tile_pool(name="mrhs", bufs=2) as mrhsp,
            tc.tile_pool(name="expp", bufs=34) as expp,
            tc.tile_pool(name="attnp", bufs=6) as attnp,
            tc.tile_pool(name="rp", bufs=5) as rp,
            tc.tile_pool(name="avp", bufs=3) as avp,
            tc.tile_pool(name="avsp", bufs=3) as avsp,
            tc.tile_pool(name="osb", bufs=4) as osb,
            tc.tile_pool(name="psl", bufs=2, space="PSUM") as psl,
        ):
            LO, HI = ds(0, DH), ds(DH, DH)   # row halves for K=64 tiling
            wq_sb = persist.tile([P, KD * JC], MM, name="wq")
            bq_sb = persist.tile([1, JC], MM, name="bq")
            wk_sb = persist.tile([P, KD * JC], MM, name="wk")
            wv_sb = persist.tile([P, KD * JC], MM, name="wv")
            bk_sb = persist.tile([1, JC], MM, name="bk")
            bv_sb = persist.tile([1, JC], MM, name="bv")

            # ---- head: one batched DMA per tensor (DGE issue slots are the
            # head bottleneck at ~0.65us each), first-needed tensors first,
            # spread over the 3 DMA-capable queues (sync/gpsimd/scalar).
            mts = {}

            # wk/mt0 chunked + pairwise round-robined over the 3 queues:
            # chunk k of both lands just before the k-projection consumes it
            QS = [nc.sync, nc.gpsimd, nc.scalar]
            mt0 = mrhsp.tile([P, KD * FB], MM, name="mchunk")
            mts[0] = mt0
            for k in range(KD):
                QS[(2 * k) % 3].dma_start(
                    wk_sb[:, ts(k, JC)], wkd[ds(k * P, P), :])
                QS[(2 * k + 1) % 3].dma_start(
                    mt0[:, ts(k, FB)], mTd[ds(k * P, P), ts(0, FB)])
                QS[(2 * k + 2) % 3].dma_start(
                    wv_sb[:, ts(k, JC)], wvd[ds(k * P, P), :])
            # memory bias, t on partitions, one column per t-chunk (f32)
            mb_sb = consts.tile([P, NTC], F32)
            nc.gpsimd.dma_start(mb_sb[:], mbd[:])

            # ---- constants ----
            # warm-up matmul operands (values irrelevant): first in the
            # DVE queue so the PE can start the moment the preamble ends
            dsrc = consts.tile([1, P + FB], MM)
            nc.vector.memset(dsrc[:], 1.0)
            ones_f = consts.tile([1, FB], F32)
            nc.vector.memset(ones_f[:], 1.0)
            ones_row = consts.tile([1, FB], MM)      # rhs for bias matmuls
            nc.vector.tensor_copy(ones_row[:], ones_f[:])
            ones_col = consts.tile([1, P], MM)       # lhsT for R bcast
            nc.vector.tensor_copy(ones_col[:], ones_f[:, 0:P])

            # ---- persistent activations ----
            # qT/kT: per head-pair tile [128 (2 heads x 64 dh), L]
            qTp = [persist.tile([P, LQ], MM, name=f"qTp{i}") for i in range(2)]
            kTp = [persist.tile([P, LM], MM, name=f"kTp{i}") for i in range(2)]
            # v: per t-chunk [128 t, 4 heads x (64 v cols + ones col + pad)]
            v_sb = [vpool.tile([P, HPC * VS], MM, name=f"v{t}")
                    for t in range(NTC)]
            # constant denominator columns (written once)
            for t in range(NTC):
                dst = v_sb[t].rearrange("p (h c) -> p h c", h=HPC)
                nc.vector.memset(dst[:, :, ds(DH, 1)], 1.0)
            wop = [persist.tile([P, D], MM, name=f"wop{i}")
                   for i in range(2)]

            if with_biases:
                nc.gpsimd.dma_start(bk_sb[:], bkd[:])
                nc.gpsimd.dma_start(bv_sb[:], bvd[:])

            def late_weight_loads():
                # issued after round 0's input chunks so the big wq/wo
                # transfers don't block the first k-projection loads
                h_ = KD // 2
                nc.sync.dma_start(
                    wq_sb[:, 0:h_ * JC].rearrange("p (k j) -> p k j",
                                                  k=h_),
                    wqd[ds(0, h_ * P), :].rearrange("(k p) j -> p k j",
                                                    p=P))
                nc.gpsimd.dma_start(
                    wq_sb[:, h_ * JC:].rearrange("p (k j) -> p k j",
                                                 k=h_),
                    wqd[ds(h_ * P, h_ * P), :].rearrange(
                        "(k p) j -> p k j", p=P))
                if with_biases:
                    nc.gpsimd.dma_start(bq_sb[:], bqd[:])
                for i in range(2):
                    nc.gpsimd.dma_start(wop[i][:], wod[ds(i * P, P), :])

            exps = {}      # (fb, hp, tc) -> [128, 2*FB] f16 tile (h2 cols)
            apairs = {}    # (fb, hp) -> [128, FB] attn pair tile
            work_q = []    # FIFO of AV units: [fb, hp, h2, tc_next, psum]
            op_q = []      # priority queue: [ready_tick, fb, fc]
            avail = {}     # (fb, hp) -> highest t-chunk with exp emitted

            def kq_proj(w_sb, b_sb, mt, dstp, col, pool=None, tag="pls",
                        fill=None):
                pool = pool if pool is not None else psl
                for hp in range(2):
                    if fill is not None:
                        fill()
                    ps = pool.tile([P, FB], F32, name="pp", tag=tag)
                    for k in range(KD):
                        w = ds(k * JC + hp * P, P)
                        nc.tensor.matmul(
                            ps[:], w_sb[:, w], mt[:, ts(k, FB)],
                            start=(k == 0),
                            stop=(not with_biases and k == KD - 1))
                    if with_biases:
                        nc.tensor.matmul(
                            ps[:], b_sb[:, ds(hp * P, P)],
                            ones_row[:], start=False, stop=True)
                    nc.vector.tensor_copy(dstp[hp][:, col], ps[:])

            def q_load(fb):
                qt = mrhsp.tile([P, KD * FB], MM, name="qchunk")
                for k in range(KD):
                    eng = nc.sync if k % 2 == 0 else nc.gpsimd
                    eng.dma_start(qt[:, ts(k, FB)],
                                  qTd[ds(k * P, P), ts(fb, FB)])
                return qt

            def q_proj(fb, qt=None, pool=None, tag="pls", fill=None):
                if qt is None:
                    qt = q_load(fb)
                kq_proj(wq_sb, bq_sb, qt, qTp, ts(fb, FB),
                        pool=pool, tag=tag, fill=fill)

            tick = [0]
            deferred = []  # [ready_tick, emit_fn] in append order

            pair_state = {}   # (fb, hp) -> [rrp [2,FB], avsp [128,FB]]

            def finish_unit_dve(u):
                # DVE-only part, emitted the moment the unit's AV chain
                # completes (never blocks the in-order PE stream)
                fb, hp, h2, _, av = u
                if h2 == 0:
                    pair_state[(fb, hp)] = [
                        [None, None],
                        avsp.tile([P, FB], F32, name="avsp")]
                rrows, avs = pair_state[(fb, hp)]
                # denominator (rb-critical) chain first, so the pair's
                # recip rows are ready well before the deferred rb
                # broadcast matmul.  A TT may only read one PSUM operand,
                # and the custom recip op misreads base_partition>0
                # sources, hence the bounces.
                avh = avp.tile([VW, FB], F32, name="avh")
                nc.vector.tensor_copy(avh[:], av[0:VW, FB:2 * FB])
                dn = rp.tile([1, FB], F32, name="dn")
                nc.vector.tensor_tensor(
                    dn[:], av[ds(DH, 1), 0:FB], avh[ds(DH, 1), :],
                    op=mybir.AluOpType.add)
                rf = rp.tile([1, FB], F32, name="rf")
                nc.vector.reciprocal_approx_fast(rf[:], dn[:])
                rrow = rp.tile([1, FB], MM, name="rrow")
                nc.vector.tensor_copy(rrow[:], rf[:])
                rrows[h2] = rrow
                # numerator halves merge into the shared pair tile
                nc.vector.tensor_tensor(
                    avs[ds(h2 * DH, DH), :], av[0:DH, 0:FB], avh[0:DH, :],
                    op=mybir.AluOpType.add)

            def finish_pair_pe(fb, hp):
                # two col-tiled K=1 matmuls broadcast the heads' recip
                # rows into the partition halves of one PSUM tile; one TT
                # (SBUF numerators x PSUM broadcast) normalizes the pair
                rrows, avs = pair_state.pop((fb, hp))
                rb_ps = psl.tile([P, 2 * FB], F32, name="rbps", tag="pls")
                for h2 in range(2):
                    nc.tensor.matmul(
                        rb_ps[ds(h2 * DH, DH), 0:FB],
                        ones_col[:, 0:DH], rrows[h2][:],
                        start=True, stop=True)
                apair = attnp.tile([P, FB], MM, name="apair")
                apairs[(fb, hp)] = apair
                nc.vector.tensor_tensor(
                    apair[:], avs[:], rb_ps[:, 0:FB],
                    op=mybir.AluOpType.mult)
                if hp == 1:
                    for fc in range(4):
                        op_q.append([tick[0] + 2, fb, fc])

            def out_proj_piece(fb, fc):
                # ops tiles come from the psx ring: its entries are units
                # finished >=1 step ago (read only by fast DVE TTs), so
                # the allocation never waits on a fresh exp the way the
                # 2-slot psl ring would
                attn = [apairs[(fb, 0)], apairs[(fb, 1)]]
                o = osb.tile([P, D], MM, name="osb")
                for jb in range(2):
                    ops = psl.tile([P, 2 * FB], F32, name="ops", tag="pls")
                    for hp in range(2):
                        nc.tensor.matmul(
                            ops[:, 0:FB], attn[hp][:, ds(fc * P, P)],
                            wop[hp][:, ts(jb, FB)],
                            start=(hp == 0), stop=(hp == 1))
                    if fb == NFB - 1:
                        # tail pieces: the ScalarE is idle once the last
                        # exp is done, while the DVE still runs the
                        # finish chains -- copy there instead
                        nc.scalar.copy(o[:, ts(jb, FB)], ops[:, 0:FB])
                    else:
                        nc.vector.tensor_copy(o[:, ts(jb, FB)],
                                              ops[:, 0:FB])
                # split the writeback across queues so the final pieces
                # drain fast at the tail
                for jb in range(2):
                    eng = QS[(2 * fc + jb) % 3]
                    eng.dma_start(
                        outd[ds(fb * FB + fc * P, P), ts(jb, FB)],
                        o[:, ts(jb, FB)])
                if fc == 3:
                    apairs.pop((fb, 0))
                    apairs.pop((fb, 1))

            def pre_drain(max_ops=1, reserve=4):
                # runs at step START, before the step's pls allocations:
                # the rbps/ops psum allocs then reuse ring entries that
                # are >=2 steps old instead of stalling on a fresh exp
                tick[0] += 1
                # out-proj pieces first: their inputs are a step old, so
                # they absorb the DVE latency of any pair-finish chain
                # whose rb matmul pops from `deferred` right after
                while (op_q and op_q[0][0] <= tick[0] and max_ops > 0
                       and len(op_q) > reserve):
                    _, fb, fc = op_q.pop(0)
                    out_proj_piece(fb, fc)
                    max_ops -= 1
                while deferred and deferred[0][0] <= tick[0]:
                    deferred.pop(0)[1]()

            def drain_av(budget):
                while budget > 0 and work_q:
                    u = work_q[0]
                    fb, hp, h2, tcn, av = u
                    if tcn >= avail.get((fb, hp), 0):
                        break   # strict FIFO; head not yet runnable
                    h = 2 * hp + h2
                    e = exps[(fb, hp, tcn)]
                    hcol = ds(h2 * FB, FB)
                    nc.tensor.matmul(
                        av[0:VW, 0:FB], v_sb[tcn][LO, ds(h * VS, VW)],
                        e[LO, hcol],
                        start=(tcn == 0), stop=(tcn == NTC - 1))
                    nc.tensor.matmul(
                        av[0:VW, FB:2 * FB], v_sb[tcn][HI, ds(h * VS, VW)],
                        e[HI, hcol],
                        start=(tcn == 0), stop=(tcn == NTC - 1))
                    u[3] += 1
                    budget -= 1
                    if u[3] == NTC:
                        work_q.pop(0)
                        finish_unit_dve(u)
                        if h2 == 1:
                            deferred.append(
                                [tick[0] + 3,
                                 lambda f=fb, p=hp: finish_pair_pe(f, p)])

            def logits_step(fb, hp, g, av_budget=4):
                # AV pairs interleave around the chunk emissions: they
                # touch no PSUM ring slots, so they fill the PE while
                # the exps gating the pls slot reallocations (and the
                # rb/ops allocs) finish
                drain_av(av_budget // 2)
                pre_drain()
                # one PSUM tile per t-chunk, both heads of the pair in
                # the two column halves; the K=64 matmuls are row-tiled
                # (partitions 0-63 / 64-127) and stream concurrently
                for s in range(G):
                    t = g * G + s
                    ps = psl.tile([P, 2 * FB], F32, name="pls", tag="pls")
                    for h2 in range(2):
                        hrow = ds(h2 * DH, DH)
                        nc.tensor.matmul(
                            ps[:, ts(h2, FB)],
                            kTp[hp][hrow, ts(t, P)],
                            qTp[hp][hrow, ts(fb, FB)],
                            start=True, stop=True)
                    e = expp.tile([P, 2 * FB], MM, name="exps")
                    nc.scalar.activation(
                        e[:], ps[:], mybir.ActivationFunctionType.Exp,
                        bias=mb_sb[:, ds(t, 1)])
                    exps[(fb, hp, t)] = e
                    if s == 0:
                        drain_av(av_budget - av_budget // 2)
                # AV consumption lags exp emission by one step so the
                # in-order PE stream never reaches an AV whose exp the
                # producer engine hasn't finished yet
                avail[(fb, hp)] = g * G

            def enqueue_block(fb, hp):
                # all previously-emitted exps are at least a step old
                for key in avail:
                    avail[key] = NTC
                for h2 in range(2):
                    av = psx.tile([P, 2 * FB], F32, name="av", tag="av")
                    work_q.append([fb, hp, h2, 0, av])

            # ---- phase 1: k/v rounds + fb0 logits ----
            # pkq keeps the k/q projection accumulators (and the warm-up
            # dummies) off the pls ring, so the phase-1 pls slots recycle
            # purely against the exp stream
            with (
                tc.tile_pool(name="ppv", bufs=2, space="PSUM") as ppv,
                tc.tile_pool(name="pkq", bufs=2, space="PSUM") as pkq,
            ):
                # PE p-state warm-up: dependency-free dummy matmuls keep
                # the PE busy across the initial DMA latency window so
                # the clock is ramping rather than idle-cold
                dum = pkq.tile([P, FB], F32, name="pp", tag="kq")
                for i in range(5):
                    nc.tensor.matmul(dum[:], dsrc[:, 0:P], dsrc[:, P:],
                                     start=True, stop=True)
                def load_mt(rnd):
                    mt = mrhsp.tile([P, KD * FB], MM, name="mchunk")
                    for k in range(KD):
                        eng = nc.sync if k % 2 == 0 else nc.gpsimd
                        eng.dma_start(mt[:, ts(k, FB)],
                                      mTd[ds(k * P, P), ts(rnd, FB)])
                    mts[rnd] = mt

                qts = {}
                for rnd in range(NFB):
                    mt = mts[rnd]
                    if rnd == 0:
                        late_weight_loads()
                    kq_proj(wk_sb, bk_sb, mt, kTp, ts(rnd, FB),
                            pool=pkq, tag="kq")
                    if rnd == 0:
                        q_proj(0, pool=pkq, tag="kq")
                    lsteps = [(g, hp) for g in (2 * rnd, 2 * rnd + 1)
                              for hp in range(2)]
                    for s in range(4):
                        g, hp = lsteps[s]
                        logits_step(0, hp, g)
                        # prefetch next round's memory chunks one round
                        # early so the k/v projections never wait on DMA
                        if s == 0 and rnd < NFB - 1:
                            load_mt(rnd + 1)
                        if s == 1 and rnd == NFB - 1:
                            qts[1] = q_load(1)
                        t = rnd * 4 + s
                        psv = ppv.tile([P, JC], F32)
                        for k in range(KD):
                            nc.tensor.matmul(
                                psv[:], mt[:, ds(k * FB + s * P, P)],
                                wv_sb[:, ts(k, JC)],
                                start=(k == 0),
                                stop=(not with_biases and k == KD - 1))
                        if with_biases:
                            nc.tensor.matmul(
                                psv[:], ones_col[:], bv_sb[:],
                                start=False, stop=True)
                        dst = v_sb[t].rearrange("p (h c) -> p h c", h=HPC)
                        nc.vector.tensor_copy(
                            dst[:, :, 0:DH],
                            psv[:].rearrange("p (h c) -> p h c", h=HPC))

            # ---- steady state: fb blocks with AV drained in-stream ----
            with tc.tile_pool(name="psx", bufs=2, space="PSUM") as psx:
                for hp in range(2):
                    enqueue_block(0, hp)
                q_proj(1, qt=qts.pop(1), fill=lambda: drain_av(3))
                for fb in range(1, NFB):
                    for hp in range(2):
                        enqueue_block(fb, hp)
                        for g in range(NTC // G):
                            logits_step(fb, hp, g,
                                        av_budget=8 if fb < 3 else 12)
                            if hp == 0 and g == 3 and fb < NFB - 1:
                                qts[fb + 1] = q_load(fb + 1)
                        if hp == 0 and fb < NFB - 1:
                            q_proj(fb + 1, qt=qts.pop(fb + 1),
                                   fill=lambda: drain_av(3))
                for key in avail:
                    avail[key] = NTC
                while deferred or work_q or op_q:
                    pre_drain(max_ops=2, reserve=0)
                    drain_av(8)

    nc.compile()
    return nc


_CACHE = {}


def _get_module(with_biases=False):
    key = ("nc", with_biases)
    if key not in _CACHE:
        _CACHE[key] = build_kernel(with_biases=with_biases)
    return _CACHE[key]


def make_in_maps(query, memory, memory_bias, Wq, bq, Wk, bk, Wv, bv, Wo, bo,
                 mm_np=None, with_biases=False):
    if mm_np is None:
        mm_np = np.float16
    query = np.asarray(query, np.float32)
    memory = np.asarray(memory, np.float32)
    memory_bias = np.asarray(memory_bias, np.float32)
    Wq = np.asarray(Wq, np.float32)
    bq = np.asarray(bq, np.float32)
    Wk = np.asarray(Wk, np.float32)
    bk = np.asarray(bk, np.float32)
    Wv = np.asarray(Wv, np.float32)
    bv = np.asarray(bv, np.float32)
    Wo = np.asarray(Wo, np.float32)
    s = np.float32(DH ** -0.5)

    qT = [np.ascontiguousarray(query[b].T).astype(mm_np) for b in range(B)]
    mT = [np.ascontiguousarray(memory[b].T).astype(mm_np) for b in range(B)]
    in_maps = []
    for c in range(NCORE):
        b, g = divmod(c, 4)
        J = slice(g * JC, (g + 1) * JC)
        m = {
            "qT": qT[b],
            "mT": mT[b],
            "wq": (np.ascontiguousarray(Wq[:, J]) * s).astype(mm_np),
            "wk": np.ascontiguousarray(Wk[:, J]).astype(mm_np),
            "wv": np.ascontiguousarray(Wv[:, J]).astype(mm_np),
            "wo": np.ascontiguousarray(Wo[J, :]).astype(mm_np),
            # mb[p, tc] = memory_bias[tc*128 + p]
            "mb": np.ascontiguousarray(
                memory_bias[b].reshape(NTC, P).T.astype(np.float32)),
        }
        if with_biases:
            m["bq"] = (bq[J] * s).reshape(1, JC).astype(mm_np)
            m["bk"] = bk[J].reshape(1, JC).astype(mm_np)
            m["bv"] = bv[J].reshape(1, JC).astype(mm_np)
        in_maps.append(m)
    return in_maps


def gather_output(results, bo):
    bo = np.asarray(bo, np.float32)
    out = np.empty((B, LQ, D), np.float32)
    for b in range(B):
        acc = results[4 * b]["out"].astype(np.float32)
        for g in range(1, 4):
            acc = acc + results[4 * b + g]["out"]
        out[b] = acc + bo
    return out


def kernel(**inputs):
    wb = any(np.any(np.asarray(inputs[b])) for b in ("bq", "bk", "bv"))
    nc = _get_module(with_biases=wb)
    in_maps = make_in_maps(**inputs, with_biases=wb)
    res = bass_utils.run_bass_kernel_spmd(nc, in_maps,
                                          core_ids=list(range(NCORE)))
    return gather_output(res.results, inputs["bo"])


# revision 62
# speedup vs baseline: 1.0016x; 1.0016x over previous
"""Multi-head attention (B=2, L=2048, D=1024, H=16) on 8 TRN2 NeuronCores.

Sharding: batch (2) x head-group (4 heads each) = 8 shards.
Each core computes q/k/v projections for its 4 heads, attention, and a
partial output projection (its 256 rows of Wo); host sums the 4 partials
per batch and adds bo.

Device dataflow (per core):
  inputs (host-prepped, f16 except memory-bias):
    qT  [1024, 2048]  = query[b].T          (d on partitions for matmul)
    mT  [1024, 2048]  = memory[b].T
    wq  [1024, 256]   = Wq[:, J] * 0.125    (scale folded)
    wk, wv [1024, 256];  bq*0.125, bk, bv [1, 256]
    wo  [256, 1024]   = Wo[J, :]
    mb  [128, 16]     = memory_bias[b] chunked (f32, t on partitions)
  phase 1: qT_h [j, f], kT_h [j, t] (head-dim on partitions), v [t, j]
           (natural) with a constant ones denominator column per head:
           softmax(s + b) @ v == (exp(s + b) @ [v | 1]) split / last-col,
           where the bias b enters through the activation's per-partition
           bias operand (t is on partitions in the logits tile).
  phase 2: one PSUM tile [128, 2*FB] per t-chunk holds both heads of a
           pair side by side; the two K=64 logits matmuls are row-tiled
           (tile_position rows 0-63 / 64-127) so they stream through the
           PE array concurrently (the pair fuses into ~one matmul time).
           exp runs on ScalarE with the memory bias applied through the
           activation's per-partition bias operand.  AV matmuls are
           split-K row-tiled pairs (t rows 0-63 / 64-127 of each chunk)
           accumulating into two PSUM column halves; the normalization
           merges the halves on the DVE, computes both heads' approx
           reciprocals, broadcasts them with two col-tiled K=1 matmuls
           into one PSUM tile, and normalizes the whole head pair with
           a single tensor_tensor.  Out-projection pieces are held in
           reserve to fill the tail while the last finish chains drain.
"""

import numpy as np

import concourse.bass as bass
import concourse.tile as tile
from concourse import bacc, mybir
from concourse import bass_utils
from concourse.bass import ts, ds

F32 = mybir.dt.float32
I16 = mybir.dt.int16
BF16 = mybir.dt.bfloat16
FP16 = mybir.dt.float16

B, LQ, LM, D, H = 2, 2048, 2048, 1024, 16
DH = 64
HPC = 4            # heads per core
JC = HPC * DH      # 256 projection cols per core
NCORE = 8
P = 128
FB = 512           # f-block width
NFB = LQ // FB     # 4
NTC = LM // P      # 16 t-chunks
KD = D // P        # 8 contraction chunks for projections
G = 2              # t-chunks per logits step

VW = DH + 1        # v columns per head incl. denominator column
VS = DH + 2        # v column stride per head (4B alignment in f16)

# Schraudolph f16 exp constants: bits = round(z * SCH_A + SCH_B + SCH_A * b)
SCH_A = 1024.0 / float(np.log(2.0))
SCH_B = 15.0 * 1024.0 - 59.297


def sch_offload(fb, tc):
    """Which exp tiles run on the DVE instead of ScalarE."""
    # With the PE as the bottleneck the ScalarE has slack, and a DVE exp
    # (deep in-order queue) only delays the pls-slot recycling; keep all
    # exp on the ScalarE.
    return False


def build_kernel(mm_dt=FP16, with_biases=False):
    MM = mm_dt
    nc = bacc.Bacc("TRN2", target_bir_lowering=False, debug=False)

    qTd = nc.dram_tensor("qT", [D, LQ], MM, kind="ExternalInput").ap()
    mTd = nc.dram_tensor("mT", [D, LM], MM, kind="ExternalInput").ap()
    wqd = nc.dram_tensor("wq", [D, JC], MM, kind="ExternalInput").ap()
    wkd = nc.dram_tensor("wk", [D, JC], MM, kind="ExternalInput").ap()
    wvd = nc.dram_tensor("wv", [D, JC], MM, kind="ExternalInput").ap()
    if with_biases:
        bqd = nc.dram_tensor("bq", [1, JC], MM, kind="ExternalInput").ap()
        bkd = nc.dram_tensor("bk", [1, JC], MM, kind="ExternalInput").ap()
        bvd = nc.dram_tensor("bv", [1, JC], MM, kind="ExternalInput").ap()
    wod = nc.dram_tensor("wo", [JC, D], MM, kind="ExternalInput").ap()
    mbd = nc.dram_tensor("mb", [P, NTC], F32, kind="ExternalInput").ap()
    outd = nc.dram_tensor("out", [LQ, D], MM, kind="ExternalOutput").ap()

    with tile.TileContext(nc) as tc:
        with (
            tc.tile_pool(name="persist", bufs=1) as persist,
            tc.tile_pool(name="vpool", bufs=1) as vpool,
            tc.tile_pool(name="consts", bufs=1) as consts,
            tc.tile_pool(name="mrhs", bufs=2) as mrhsp,
            tc.tile_pool(name="expp", bufs=34) as expp,
            tc.tile_pool(name="attnp", bufs=6) as attnp,
            tc.tile_pool(name="rp", bufs=5) as rp,
            tc.tile_pool(name="avp", bufs=3) as avp,
            tc.tile_pool(name="avsp", bufs=3) as avsp,
            tc.tile_pool(name="osb", bufs=4) as osb,
            tc.tile_pool(name="psl", bufs=2, space="PSUM") as psl,
        ):
            LO, HI = ds(0, DH), ds(DH, DH)   # row halves for K=64 tiling
            wq_sb = persist.tile([P, KD * JC], MM, name="wq")
            bq_sb = persist.tile([1, JC], MM, name="bq")
            wk_sb = persist.tile([P, KD * JC], MM, name="wk")
            wv_sb = persist.tile([P, KD * JC], MM, name="wv")
            bk_sb = persist.tile([1, JC], MM, name="bk")
            bv_sb = persist.tile([1, JC], MM, name="bv")

            # ---- head: one batched DMA per tensor (DGE issue slots are the
            # head bottleneck at ~0.65us each), first-needed tensors first,
            # spread over the 3 DMA-capable queues (sync/gpsimd/scalar).
            mts = {}

            # wk/mt0 chunked + pairwise round-robined over the 3 queues:
            # chunk k of both lands just before the k-projection consumes it
            QS = [nc.sync, nc.gpsimd, nc.scalar]
            mt0 = mrhsp.tile([P, KD * FB], MM, name="mchunk")
            mts[0] = mt0
            for k in range(KD):
                QS[(2 * k) % 3].dma_start(
                    wk_sb[:, ts(k, JC)], wkd[ds(k * P, P), :])
                QS[(2 * k + 1) % 3].dma_start(
                    mt0[:, ts(k, FB)], mTd[ds(k * P, P), ts(0, FB)])
                QS[(2 * k + 2) % 3].dma_start(
                    wv_sb[:, ts(k, JC)], wvd[ds(k * P, P), :])
            # memory bias, t on partitions, one column per t-chunk (f32)
            mb_sb = consts.tile([P, NTC], F32)
            nc.gpsimd.dma_start(mb_sb[:], mbd[:])

            # ---- constants ----
            # warm-up matmul operands (values irrelevant): first in the
            # DVE queue so the PE can start the moment the preamble ends
            dsrc = consts.tile([1, P + FB], MM)
            nc.vector.memset(dsrc[:], 1.0)
            ones_f = consts.tile([1, FB], F32)
            nc.vector.memset(ones_f[:], 1.0)
            ones_row = consts.tile([1, FB], MM)      # rhs for bias matmuls
            nc.vector.tensor_copy(ones_row[:], ones_f[:])
            ones_col = consts.tile([1, P], MM)       # lhsT for R bcast
            nc.vector.tensor_copy(ones_col[:], ones_f[:, 0:P])

            # ---- persistent activations ----
            # qT/kT: per head-pair tile [128 (2 heads x 64 dh), L]
            qTp = [persist.tile([P, LQ], MM, name=f"qTp{i}") for i in range(2)]
            kTp = [persist.tile([P, LM], MM, name=f"kTp{i}") for i in range(2)]
            # v: per t-chunk [128 t, 4 heads x (64 v cols + ones col + pad)]
            v_sb = [vpool.tile([P, HPC * VS], MM, name=f"v{t}")
                    for t in range(NTC)]
            # constant denominator columns (written once)
            for t in range(NTC):
                dst = v_sb[t].rearrange("p (h c) -> p h c", h=HPC)
                nc.vector.memset(dst[:, :, ds(DH, 1)], 1.0)
            wop = [persist.tile([P, D], MM, name=f"wop{i}")
                   for i in range(2)]

            if with_biases:
                nc.gpsimd.dma_start(bk_sb[:], bkd[:])
                nc.gpsimd.dma_start(bv_sb[:], bvd[:])

            def late_weight_loads():
                # issued after round 0's input chunks so the big wq/wo
                # transfers don't block the first k-projection loads
                h_ = KD // 2
                nc.sync.dma_start(
                    wq_sb[:, 0:h_ * JC].rearrange("p (k j) -> p k j",
                                                  k=h_),
                    wqd[ds(0, h_ * P), :].rearrange("(k p) j -> p k j",
                                                    p=P))
                nc.gpsimd.dma_start(
                    wq_sb[:, h_ * JC:].rearrange("p (k j) -> p k j",
                                                 k=h_),
                    wqd[ds(h_ * P, h_ * P), :].rearrange(
                        "(k p) j -> p k j", p=P))
                if with_biases:
                    nc.gpsimd.dma_start(bq_sb[:], bqd[:])
                for i in range(2):
                    nc.gpsimd.dma_start(wop[i][:], wod[ds(i * P, P), :])

            exps = {}      # (fb, hp, tc) -> [128, 2*FB] f16 tile (h2 cols)
            apairs = {}    # (fb, hp) -> [128, FB] attn pair tile
            work_q = []    # FIFO of AV units: [fb, hp, h2, tc_next, psum]
            op_q = []      # priority queue: [ready_tick, fb, fc]
            avail = {}     # (fb, hp) -> highest t-chunk with exp emitted

            def kq_proj(w_sb, b_sb, mt, dstp, col, pool=None, tag="pls",
                        fill=None):
                pool = pool if pool is not None else psl
                for hp in range(2):
                    if fill is not None:
                        fill()
                    ps = pool.tile([P, FB], F32, name="pp", tag=tag)
                    for k in range(KD):
                        w = ds(k * JC + hp * P, P)
                        nc.tensor.matmul(
                            ps[:], w_sb[:, w], mt[:, ts(k, FB)],
                            start=(k == 0),
                            stop=(not with_biases and k == KD - 1))
                    if with_biases:
                        nc.tensor.matmul(
                            ps[:], b_sb[:, ds(hp * P, P)],
                            ones_row[:], start=False, stop=True)
                    nc.vector.tensor_copy(dstp[hp][:, col], ps[:])

            def q_load(fb):
                qt = mrhsp.tile([P, KD * FB], MM, name="qchunk")
                for k in range(KD):
                    eng = nc.sync if k % 2 == 0 else nc.gpsimd
                    eng.dma_start(qt[:, ts(k, FB)],
                                  qTd[ds(k * P, P), ts(fb, FB)])
                return qt

            def q_proj(fb, qt=None, pool=None, tag="pls", fill=None):
                if qt is None:
                    qt = q_load(fb)
                kq_proj(wq_sb, bq_sb, qt, qTp, ts(fb, FB),
                        pool=pool, tag=tag, fill=fill)

            tick = [0]
            deferred = []  # [ready_tick, emit_fn] in append order

            pair_state = {}   # (fb, hp) -> [rrp [2,FB], avsp [128,FB]]

            def finish_unit_dve(u):
                # DVE-only part, emitted the moment the unit's AV chain
                # completes (never blocks the in-order PE stream)
                fb, hp, h2, _, av = u
                if h2 == 0:
                    pair_state[(fb, hp)] = [
                        [None, None],
                        avsp.tile([P, FB], F32, name="avsp"), []]
                rrows, avs, slowq = pair_state[(fb, hp)]
                # denominator (rb-critical) chain first, so the pair's
                # recip rows are ready well before the deferred rb
                # broadcast matmul.  A TT may only read one PSUM operand,
                # and the custom recip op misreads base_partition>0
                # sources, hence the bounces.
                last = (fb == NFB - 1 and hp == 1)
                if last:
                    # tail: keep the rb-critical denominator chain ahead
                    # of BOTH units' numerator merges in the in-order DVE
                    # queue, so the final reciprocal lands ~1.3us earlier
                    dnh = rp.tile([1, FB], F32, name="dn")
                    nc.vector.tensor_copy(dnh[:], av[ds(DH, 1), FB:2 * FB])
                    dn = rp.tile([1, FB], F32, name="dn")
                    nc.vector.tensor_tensor(
                        dn[:], av[ds(DH, 1), 0:FB], dnh[:],
                        op=mybir.AluOpType.add)
                else:
                    avh = avp.tile([VW, FB], F32, name="avh")
                    nc.vector.tensor_copy(avh[:], av[0:VW, FB:2 * FB])
                    dn = rp.tile([1, FB], F32, name="dn")
                    nc.vector.tensor_tensor(
                        dn[:], av[ds(DH, 1), 0:FB], avh[ds(DH, 1), :],
                        op=mybir.AluOpType.add)
                rf = rp.tile([1, FB], F32, name="rf")
                nc.vector.reciprocal_approx_fast(rf[:], dn[:])
                rrow = rp.tile([1, FB], MM, name="rrow")
                nc.vector.tensor_copy(rrow[:], rf[:])
                rrows[h2] = rrow
                if last:
                    def slow(av=av, h2=h2, avs=avs):
                        avh = avp.tile([VW, FB], F32, name="avh")
                        nc.vector.tensor_copy(avh[:], av[0:VW, FB:2 * FB])
                        nc.vector.tensor_tensor(
                            avs[ds(h2 * DH, DH), :], av[0:DH, 0:FB],
                            avh[0:DH, :], op=mybir.AluOpType.add)
                    slowq.append(slow)
                    if h2 == 1:
                        for s in slowq:
                            s()
                else:
                    # numerator halves merge into the shared pair tile
                    nc.vector.tensor_tensor(
                        avs[ds(h2 * DH, DH), :], av[0:DH, 0:FB],
                        avh[0:DH, :], op=mybir.AluOpType.add)

            def finish_pair_pe(fb, hp):
                # two col-tiled K=1 matmuls broadcast the heads' recip
                # rows into the partition halves of one PSUM tile; one TT
                # (SBUF numerators x PSUM broadcast) normalizes the pair
                rrows, avs, _ = pair_state.pop((fb, hp))
                rb_ps = psl.tile([P, 2 * FB], F32, name="rbps", tag="pls")
                for h2 in range(2):
                    nc.tensor.matmul(
                        rb_ps[ds(h2 * DH, DH), 0:FB],
                        ones_col[:, 0:DH], rrows[h2][:],
                        start=True, stop=True)
                apair = attnp.tile([P, FB], MM, name="apair")
                apairs[(fb, hp)] = apair
                nc.vector.tensor_tensor(
                    apair[:], avs[:], rb_ps[:, 0:FB],
                    op=mybir.AluOpType.mult)
                if hp == 1:
                    for fc in range(4):
                        op_q.append([tick[0] + 2, fb, fc])

            def out_proj_piece(fb, fc):
                # ops tiles come from the psx ring: its entries are units
                # finished >=1 step ago (read only by fast DVE TTs), so
                # the allocation never waits on a fresh exp the way the
                # 2-slot psl ring would
                attn = [apairs[(fb, 0)], apairs[(fb, 1)]]
                o = osb.tile([P, D], MM, name="osb")
                for jb in range(2):
                    ops = psl.tile([P, 2 * FB], F32, name="ops", tag="pls")
                    for hp in range(2):
                        nc.tensor.matmul(
                            ops[:, 0:FB], attn[hp][:, ds(fc * P, P)],
                            wop[hp][:, ts(jb, FB)],
                            start=(hp == 0), stop=(hp == 1))
                    if fb == NFB - 1:
                        # tail pieces: the ScalarE is idle once the last
                        # exp is done, while the DVE still runs the
                        # finish chains -- copy there instead
                        nc.scalar.copy(o[:, ts(jb, FB)], ops[:, 0:FB])
                    else:
                        nc.vector.tensor_copy(o[:, ts(jb, FB)],
                                              ops[:, 0:FB])
                # split the writeback across queues so the final pieces
                # drain fast at the tail
                for jb in range(2):
                    eng = QS[(2 * fc + jb) % 3]
                    eng.dma_start(
                        outd[ds(fb * FB + fc * P, P), ts(jb, FB)],
                        o[:, ts(jb, FB)])
                if fc == 3:
                    apairs.pop((fb, 0))
                    apairs.pop((fb, 1))

            def pre_drain(max_ops=1, reserve=4):
                # runs at step START, before the step's pls allocations:
                # the rbps/ops psum allocs then reuse ring entries that
                # are >=2 steps old instead of stalling on a fresh exp
                tick[0] += 1
                # out-proj pieces first: their inputs are a step old, so
                # they absorb the DVE latency of any pair-finish chain
                # whose rb matmul pops from `deferred` right after
                while (op_q and op_q[0][0] <= tick[0] and max_ops > 0
                       and len(op_q) > reserve):
                    _, fb, fc = op_q.pop(0)
                    out_proj_piece(fb, fc)
                    max_ops -= 1
                while deferred and deferred[0][0] <= tick[0]:
                    deferred.pop(0)[1]()

            def drain_av(budget):
                while budget > 0 and work_q:
                    u = work_q[0]
                    fb, hp, h2, tcn, av = u
                    if tcn >= avail.get((fb, hp), 0):
                        break   # strict FIFO; head not yet runnable
                    h = 2 * hp + h2
                    e = exps[(fb, hp, tcn)]
                    hcol = ds(h2 * FB, FB)
                    nc.tensor.matmul(
                        av[0:VW, 0:FB], v_sb[tcn][LO, ds(h * VS, VW)],
                        e[LO, hcol],
                        start=(tcn == 0), stop=(tcn == NTC - 1))
                    nc.tensor.matmul(
                        av[0:VW, FB:2 * FB], v_sb[tcn][HI, ds(h * VS, VW)],
                        e[HI, hcol],
                        start=(tcn == 0), stop=(tcn == NTC - 1))
                    u[3] += 1
                    budget -= 1
                    if u[3] == NTC:
                        work_q.pop(0)
                        finish_unit_dve(u)
                        if h2 == 1:
                            deferred.append(
                                [tick[0] + 3,
                                 lambda f=fb, p=hp: finish_pair_pe(f, p)])

            def logits_step(fb, hp, g, av_budget=4):
                # AV pairs interleave around the chunk emissions: they
                # touch no PSUM ring slots, so they fill the PE while
                # the exps gating the pls slot reallocations (and the
                # rb/ops allocs) finish
                drain_av(av_budget // 2)
                pre_drain()
                # one PSUM tile per t-chunk, both heads of the pair in
                # the two column halves; the K=64 matmuls are row-tiled
                # (partitions 0-63 / 64-127) and stream concurrently
                for s in range(G):
                    t = g * G + s
                    ps = psl.tile([P, 2 * FB], F32, name="pls", tag="pls")
                    for h2 in range(2):
                        hrow = ds(h2 * DH, DH)
                        nc.tensor.matmul(
                            ps[:, ts(h2, FB)],
                            kTp[hp][hrow, ts(t, P)],
                            qTp[hp][hrow, ts(fb, FB)],
                            start=True, stop=True)
                    e = expp.tile([P, 2 * FB], MM, name="exps")
                    nc.scalar.activation(
                        e[:], ps[:], mybir.ActivationFunctionType.Exp,
                        bias=mb_sb[:, ds(t, 1)])
                    exps[(fb, hp, t)] = e
                    if s == 0:
                        drain_av(av_budget - av_budget // 2)
                # AV consumption lags exp emission by one step so the
                # in-order PE stream never reaches an AV whose exp the
                # producer engine hasn't finished yet
                avail[(fb, hp)] = g * G

            def enqueue_block(fb, hp):
                # all previously-emitted exps are at least a step old
                for key in avail:
                    avail[key] = NTC
                for h2 in range(2):
                    av = psx.tile([P, 2 * FB], F32, name="av", tag="av")
                    work_q.append([fb, hp, h2, 0, av])

            # ---- phase 1: k/v rounds + fb0 logits ----
            # pkq keeps the k/q projection accumulators (and the warm-up
            # dummies) off the pls ring, so the phase-1 pls slots recycle
            # purely against the exp stream
            with (
                tc.tile_pool(name="ppv", bufs=2, space="PSUM") as ppv,
                tc.tile_pool(name="pkq", bufs=2, space="PSUM") as pkq,
            ):
                # PE p-state warm-up: dependency-free dummy matmuls keep
                # the PE busy across the initial DMA latency window so
                # the clock is ramping rather than idle-cold
                dum = pkq.tile([P, FB], F32, name="pp", tag="kq")
                for i in range(7):
                    nc.tensor.matmul(dum[:], dsrc[:, 0:P], dsrc[:, P:],
                                     start=True, stop=True)
                def load_mt(rnd):
                    mt = mrhsp.tile([P, KD * FB], MM, name="mchunk")
                    for k in range(KD):
                        eng = nc.sync if k % 2 == 0 else nc.gpsimd
                        eng.dma_start(mt[:, ts(k, FB)],
                                      mTd[ds(k * P, P), ts(rnd, FB)])
                    mts[rnd] = mt

                qts = {}
                for rnd in range(NFB):
                    mt = mts[rnd]
                    if rnd == 0:
                        late_weight_loads()
                    kq_proj(wk_sb, bk_sb, mt, kTp, ts(rnd, FB),
                            pool=pkq, tag="kq")
                    if rnd == 0:
                        q_proj(0, pool=pkq, tag="kq")
                    lsteps = [(g, hp) for g in (2 * rnd, 2 * rnd + 1)
                              for hp in range(2)]
                    for s in range(4):
                        g, hp = lsteps[s]
                        logits_step(0, hp, g)
                        # prefetch next round's memory chunks one round
                        # early so the k/v projections never wait on DMA
                        if s == 0 and rnd < NFB - 1:
                            load_mt(rnd + 1)
                        if s == 1 and rnd == NFB - 1:
                            qts[1] = q_load(1)
                        t = rnd * 4 + s
                        psv = ppv.tile([P, JC], F32)
                        for k in range(KD):
                            nc.tensor.matmul(
                                psv[:], mt[:, ds(k * FB + s * P, P)],
                                wv_sb[:, ts(k, JC)],
                                start=(k == 0),
                                stop=(not with_biases and k == KD - 1))
                        if with_biases:
                            nc.tensor.matmul(
                                psv[:], ones_col[:], bv_sb[:],
                                start=False, stop=True)
                        dst = v_sb[t].rearrange("p (h c) -> p h c", h=HPC)
                        nc.vector.tensor_copy(
                            dst[:, :, 0:DH],
                            psv[:].rearrange("p (h c) -> p h c", h=HPC))

            # ---- steady state: fb blocks with AV drained in-stream ----
            with tc.tile_pool(name="psx", bufs=2, space="PSUM") as psx:
                for hp in range(2):
                    enqueue_block(0, hp)
                q_proj(1, qt=qts.pop(1), fill=lambda: drain_av(3))
                for fb in range(1, NFB):
                    for hp in range(2):
                        enqueue_block(fb, hp)
                        for g in range(NTC // G):
                            logits_step(fb, hp, g,
                                        av_budget=8 if fb < 3 else 12)
                            if hp == 0 and g == 3 and fb < NFB - 1:
                                qts[fb + 1] = q_load(fb + 1)
                        if hp == 0 and fb < NFB - 1:
                            q_proj(fb + 1, qt=qts.pop(fb + 1),
                                   fill=lambda: drain_av(3))
                for key in avail:
                    avail[key] = NTC
                while deferred or work_q or op_q:
                    pre_drain(max_ops=2, reserve=0)
                    drain_av(8)

    nc.compile()
    return nc


_CACHE = {}


def _get_module(with_biases=False):
    key = ("nc", with_biases)
    if key not in _CACHE:
        _CACHE[key] = build_kernel(with_biases=with_biases)
    return _CACHE[key]


def make_in_maps(query, memory, memory_bias, Wq, bq, Wk, bk, Wv, bv, Wo, bo,
                 mm_np=None, with_biases=False):
    if mm_np is None:
        mm_np = np.float16
    query = np.asarray(query, np.float32)
    memory = np.asarray(memory, np.float32)
    memory_bias = np.asarray(memory_bias, np.float32)
    Wq = np.asarray(Wq, np.float32)
    bq = np.asarray(bq, np.float32)
    Wk = np.asarray(Wk, np.float32)
    bk = np.asarray(bk, np.float32)
    Wv = np.asarray(Wv, np.float32)
    bv = np.asarray(bv, np.float32)
    Wo = np.asarray(Wo, np.float32)
    s = np.float32(DH ** -0.5)

    qT = [np.ascontiguousarray(query[b].T).astype(mm_np) for b in range(B)]
    mT = [np.ascontiguousarray(memory[b].T).astype(mm_np) for b in range(B)]
    in_maps = []
    for c in range(NCORE):
        b, g = divmod(c, 4)
        J = slice(g * JC, (g + 1) * JC)
        m = {
            "qT": qT[b],
            "mT": mT[b],
            "wq": (np.ascontiguousarray(Wq[:, J]) * s).astype(mm_np),
            "wk": np.ascontiguousarray(Wk[:, J]).astype(mm_np),
            "wv": np.ascontiguousarray(Wv[:, J]).astype(mm_np),
            "wo": np.ascontiguousarray(Wo[J, :]).astype(mm_np),
            # mb[p, tc] = memory_bias[tc*128 + p]
            "mb": np.ascontiguousarray(
                memory_bias[b].reshape(NTC, P).T.astype(np.float32)),
        }
        if with_biases:
            m["bq"] = (bq[J] * s).reshape(1, JC).astype(mm_np)
            m["bk"] = bk[J].reshape(1, JC).astype(mm_np)
            m["bv"] = bv[J].reshape(1, JC).astype(mm_np)
        in_maps.append(m)
    return in_maps


def gather_output(results, bo):
    bo = np.asarray(bo, np.float32)
    out = np.empty((B, LQ, D), np.float32)
    for b in range(B):
        acc = results[4 * b]["out"].astype(np.float32)
        for g in range(1, 4):
            acc = acc + results[4 * b + g]["out"]
        out[b] = acc + bo
    return out


def kernel(**inputs):
    wb = any(np.any(np.asarray(inputs[b])) for b in ("bq", "bk", "bv"))
    nc = _get_module(with_biases=wb)
    in_maps = make_in_maps(**inputs, with_biases=wb)
    res = bass_utils.run_bass_kernel_spmd(nc, in_maps,
                                          core_ids=list(range(NCORE)))
    return gather_output(res.results, inputs["bo"])
